# revision 8
# baseline (speedup 1.0000x reference)
"""Trainium2 Bass kernel for nn_CATCallerEncoderLayer (dynamic-conv encoder layer).

Reference computation (T=1024, B=16, C=512, H=8, K=31, P=15):
  h  = x @ w1 + b1; a, g = split(h); xg = a * sigmoid(g)
  w  = softmax((xg @ wl_w + wl_b).reshape(T,B,H,K), axis=-1)
  out[t,b,h*64+r] = sum_k w[t,b,h,k] * xg_pad[t+k-15, b, h*64+r]
  return out @ w2 + b2

Sharding: data-parallel over B across 8 cores (2 batches/core). Host supplies
x pre-transposed to feature-major [b, C, T] and wl_w zero-padded to 256 cols.

Per core:
  mm1 (f32r, lhsT = xT slices)  -> h1 token-major PSUM -> GLU -> xg [t,C] SBUF
  PE-transpose xg (f32)         -> mm_dyn (f32r, N=256) -> w248 [t,248] PSUM
  exp / group-sum / recip       -> wsoft -> cast bf16
  M-form shear-write (bf16) to zero-filled DRAM staging:
      stage[(b,h,i)-block row t_loc, col t_loc+k] = wsoft[t, h, k]  (98-blocks)
  readback [98, 8*128] -> 8 PE-transposes (bf16) -> band.T tiles [s_pad, t]
  conv: psum[c64x2, t] += halo_bf16[:, h].T @ band_h   (banded matmul)
  mm2 (f32r, lhsT = conv feature-major tiles) -> out [t, C] -> DRAM
"""
import sys

sys.path.insert(0, "/opt/trn_rl_repo")

import numpy as np

T, B, C = 1024, 16, 512
H, KT, PAD = 8, 31, 15
HK = H * KT          # 248
HKP = 256            # wl_w padded cols
NCORES = 8
BPC = B // NCORES    # 2
BT = 98              # conv time-block (s_pad = t + k <= 127)
NBLK = (T + BT - 1) // BT  # 11
NTC = T // 128       # 8
NCC = C // 128       # 4
SBLK = 98 * 128      # staging elements per (b,h,i) block

_cache = {}


def _split_sync_waits(nc, mybir, max_waits=1):
    """This walrus build rejects instructions carrying >1 sync-wait command.
    Hoist extra waits onto same-engine NOPs inserted just before."""
    cnt = 0
    for f in nc.m.functions:
        for bb in f.blocks:
            new = []
            for inst in bb.instructions:
                si = inst.sync_info
                if si is not None and si.on_wait and len(si.on_wait) > max_waits:
                    waits = list(si.on_wait)
                    for w in waits[:-max_waits]:
                        cnt += 1
                        new.append(
                            mybir.InstNoOp(
                                name=f"I-ws{cnt}",
                                engine=inst.engine,
                                sync_info=mybir.SyncInfo(on_wait=[w], on_update=[]),
                            )
                        )
                    inst.sync_info = mybir.SyncInfo(
                        on_wait=waits[-max_waits:], on_update=list(si.on_update or [])
                    )
                new.append(inst)
            bb.instructions = new
    return cnt


def _build(has_b1, has_wlb, has_b2, reps=1):
    import bass_rust
    import concourse.bass as bass
    import concourse.tile as tile
    from concourse import mybir

    f32 = mybir.dt.float32
    f32r = mybir.dt.float32r
    bf16 = mybir.dt.bfloat16
    AF = mybir.ActivationFunctionType
    AX = mybir.AxisListType

    def r(ap):
        return ap.bitcast(f32r)

    nc = bass.Bass("TRN2", debug=False)

    xt_d = nc.dram_tensor("xt", (BPC, C, T), f32, kind="ExternalInput").ap()
    w1_d = nc.dram_tensor("w1", (C, 2 * C), f32, kind="ExternalInput").ap()
    wlw_d = nc.dram_tensor("wl_w", (C, HKP), f32, kind="ExternalInput").ap()
    w2_d = nc.dram_tensor("w2", (C, C), f32, kind="ExternalInput").ap()
    b1_d = nc.dram_tensor("b1", (2 * C,), f32, kind="ExternalInput").ap()
    wlb_d = nc.dram_tensor("wl_b", (HK,), f32, kind="ExternalInput").ap()
    b2_d = nc.dram_tensor("b2", (C,), f32, kind="ExternalInput").ap()
    eye_d = nc.dram_tensor("eye", (128, 128), f32, kind="ExternalInput").ap()
    out_d = nc.dram_tensor("out", (T, BPC, C), f32, kind="ExternalOutput").ap()
    # bf16 M-form band staging: per (b,h,i) block of [98, 128]
    n_sblk = BPC * H * NBLK
    stage = nc.dram_tensor("stage", (n_sblk * 98, 128), bf16).ap()
    stage_f = stage[:].flatten()

    def sbase(b, h, i):
        return (((b * H) + h) * NBLK + i) * SBLK

    with tile.TileContext(nc) as tc:
        with (
            tc.tile_pool(name="consts", bufs=1) as cpool,
            tc.tile_pool(name="xt", bufs=BPC * NCC) as xtpool,
            tc.tile_pool(name="xg", bufs=BPC * NTC) as xgpool,
            tc.tile_pool(name="xgb", bufs=BPC * NTC) as xgbpool,
            tc.tile_pool(name="work", bufs=3) as wpool,
            tc.tile_pool(name="xgf", bufs=6) as xgfpool,
            tc.tile_pool(name="soft", bufs=3) as spool,
            tc.tile_pool(name="mread", bufs=3) as mpool,
            tc.tile_pool(name="band", bufs=10) as bpool,
            tc.tile_pool(name="halo", bufs=3) as hpool,
            tc.tile_pool(name="outp", bufs=3) as opool,
        ):
            # ---- constants ----
            w1_t, wlw_t, w2_t = [], [], []
            for cc in range(NCC):
                tw1 = cpool.tile([128, 2 * C], f32r, tag=f"w1_{cc}")
                nc.gpsimd.dma_start(tw1[:], w1_d[cc * 128:(cc + 1) * 128, :])
                w1_t.append(tw1)
                twl = cpool.tile([128, HKP], f32r, tag=f"wlw_{cc}")
                nc.gpsimd.dma_start(twl[:], wlw_d[cc * 128:(cc + 1) * 128, :])
                wlw_t.append(twl)
                tw2 = cpool.tile([128, C], f32r, tag=f"w2_{cc}")
                nc.gpsimd.dma_start(tw2[:], w2_d[cc * 128:(cc + 1) * 128, :])
                w2_t.append(tw2)
            eye = cpool.tile([128, 128], f32, tag="eye")
            nc.sync.dma_start(eye[:], eye_d[:])
            eyeb = cpool.tile([128, 128], bf16, tag="eyeb")
            nc.vector.tensor_copy(eyeb[:], eye[:])
            if has_b1:
                b1a = cpool.tile([128, C], f32, tag="b1a")
                nc.sync.dma_start(b1a[:], b1_d[None, 0:C].to_broadcast((128, C)))
                b1g = cpool.tile([128, C], f32, tag="b1g")
                nc.sync.dma_start(b1g[:], b1_d[None, C:2 * C].to_broadcast((128, C)))
            if has_wlb:
                wlb = cpool.tile([128, HK], f32, tag="wlb")
                nc.sync.dma_start(wlb[:], wlb_d[None, :].to_broadcast((128, HK)))
            if has_b2:
                b2t = cpool.tile([128, C], f32, tag="b2t")
                nc.sync.dma_start(b2t[:], b2_d[None, :].to_broadcast((128, C)))

            # zero-fill staging (garbage-free band tiles; persists across reps)
            zt = cpool.tile([128, 2048], bf16, tag="zt")
            nc.vector.memset(zt[:], 0.0)
            nrows = n_sblk * 98
            zrows = 2048
            pos = 0
            while pos < nrows:
                n = min(zrows, nrows - pos)
                nc.scalar.dma_start(stage[pos:pos + n, :], zt[:, :n])
                pos += n

            xt_t = {}
            for b in range(BPC):
                for cc in range(NCC):
                    tx = xtpool.tile([128, T], f32r, tag="xt")
                    nc.gpsimd.dma_start(tx[:], xt_d[b, cc * 128:(cc + 1) * 128, :])
                    xt_t[(b, cc)] = tx

            for rep in range(reps):
                xg_t = {}
                xgb_t = {}

                # ================= Phase A =================
                with tc.tile_pool(name=f"psumA{rep}", bufs=2, space="PSUM") as psA:
                    for b in range(BPC):
                        for tch in range(NTC):
                            t0 = tch * 128
                            h1a = psA.tile([128, C], f32, tag="h1a")
                            h1g = psA.tile([128, C], f32, tag="h1g")
                            for cc in range(NCC):
                                lhsT = xt_t[(b, cc)][:, t0:t0 + 128]
                                nc.tensor.matmul(h1a[:], lhsT, w1_t[cc][:, 0:C],
                                                 start=(cc == 0), stop=(cc == NCC - 1))
                            for cc in range(NCC):
                                lhsT = xt_t[(b, cc)][:, t0:t0 + 128]
                                nc.tensor.matmul(h1g[:], lhsT, w1_t[cc][:, C:2 * C],
                                                 start=(cc == 0), stop=(cc == NCC - 1))
                            # GLU: xg = (a + b1a) * sigmoid(g + b1g)
                            sg = wpool.tile([128, C], f32, tag="sg")
                            if has_b1:
                                gb = wpool.tile([128, C], f32, tag="gb")
                                nc.vector.tensor_add(gb[:], h1g[:], b1g[:])
                                nc.scalar.activation(sg[:], gb[:], AF.Sigmoid)
                            else:
                                nc.scalar.activation(sg[:], h1g[:], AF.Sigmoid)
                            xg = xgpool.tile([128, C], f32, tag="xg")
                            if has_b1:
                                ab = wpool.tile([128, C], f32, tag="ab")
                                nc.vector.tensor_add(ab[:], h1a[:], b1a[:])
                                nc.vector.tensor_mul(xg[:], ab[:], sg[:])
                            else:
                                nc.vector.tensor_mul(xg[:], h1a[:], sg[:])
                            xg_t[(b, tch)] = xg
                            xgb = xgbpool.tile([128, C], bf16, tag="xgb")
                            nc.vector.tensor_copy(xgb[:], xg[:])
                            xgb_t[(b, tch)] = xgb

                            # transpose xg -> feature-major chunks, then mm_dyn
                            w248 = psA.tile([128, HKP], f32, tag="w248")
                            for cc in range(NCC):
                                tp = psA.tile([128, 128], f32, tag="tp")
                                nc.tensor.transpose(tp[:], xg[:, cc * 128:(cc + 1) * 128],
                                                    eye[:])
                                xgf = xgfpool.tile([128, 128], f32r, tag="xgf")
                                nc.scalar.copy(xgf[:], tp[:])
                                nc.tensor.matmul(w248[:], xgf[:], wlw_t[cc][:],
                                                 start=(cc == 0), stop=(cc == NCC - 1))
                            # softmax over taps (values tiny: skip max-subtract)
                            we = spool.tile([128, HK], f32, tag="we")
                            if has_wlb:
                                wb = spool.tile([128, HK], f32, tag="wb")
                                nc.vector.tensor_add(wb[:], w248[:, 0:HK], wlb[:])
                                nc.scalar.activation(we[:], wb[:], AF.Exp)
                            else:
                                nc.scalar.activation(we[:], w248[:, 0:HK], AF.Exp)
                            sums = spool.tile([128, H], f32, tag="sums")
                            we3 = we[:].rearrange("t (h k) -> t h k", k=KT)
                            nc.vector.reduce_sum(sums[:], we3, axis=AX.X)
                            rec = spool.tile([128, H], f32, tag="rec")
                            nc.vector.reciprocal(rec[:], sums[:])
                            wsb = spool.tile([128, HK], bf16, tag="wsb")
                            ws3 = wsb[:].rearrange("t (h k) -> t h k", k=KT)
                            rec3 = rec[:, :, None].to_broadcast((128, H, KT))
                            nc.vector.tensor_mul(ws3, we3, rec3)
                            # M-form shear-write to staging: all 8 heads per DMA,
                            # split by overlapped 98-blocks. Element (t,h,k) goes
                            # to sbase(b,h,i) + t_loc*129 + k.
                            g0, g1 = t0, t0 + 128
                            i = g0 // BT
                            while i < NBLK and i * BT < g1:
                                r0 = max(g0, i * BT)
                                r1 = min(g1, i * BT + BT)
                                nr = r1 - r0
                                tl0 = r0 - i * BT
                                src = wsb[r0 - g0:r1 - g0, :].rearrange(
                                    "t (h k) -> t h k", k=KT)
                                dst = stage_f[:1].copy()
                                dst.ap = bass_rust.VecI64Pair(
                                    [[129, nr], [NBLK * SBLK, H], [1, KT]])
                                dst.offset = sbase(b, 0, i) + tl0 * 129
                                nc.sync.dma_start(dst, src)
                                i += 1

                # ================= Phase B =================
                with tc.tile_pool(name=f"psumB{rep}", bufs=2, space="PSUM") as psB:
                    for b in range(BPC):
                        for i in range(NBLK):
                            t0 = i * BT
                            blk = min(BT, T - t0)
                            # halo tile: bf16 xg rows [t0-15, t0-15+128)
                            halo = hpool.tile([128, C], bf16, tag="halo")
                            lo = t0 - PAD
                            hi = lo + 128
                            vlo, vhi = max(lo, 0), min(hi, T)
                            if vlo > lo or vhi < hi:
                                nc.vector.memset(halo[:], 0.0)
                            rr = vlo
                            while rr < vhi:
                                src = xgb_t[(b, rr // 128)]
                                n = min(vhi - rr, 128 - (rr % 128))
                                nc.sync.dma_start(
                                    halo[rr - lo:rr - lo + n, :],
                                    src[rr % 128:rr % 128 + n, :],
                                )
                                rr += n
                            # readback M-form [98, 8*128] in one DMA
                            mt = mpool.tile([98, H * 128], f32, tag="mt")
                            mt3 = mt[:].rearrange("t (h s) -> t h s", s=128)
                            rd = stage_f[:1].copy()
                            rd.ap = bass_rust.VecI64Pair(
                                [[128, 98], [NBLK * SBLK, H], [1, 128]])
                            rd.offset = sbase(b, 0, i)
                            nc.gpsimd.dma_start(mt3, rd)
                            # per head: PE-transpose -> band.T, then conv matmuls
                            cp = psB.tile([128, NCC * blk], f32, tag="cp")
                            for j in range(NCC):
                                for jj in range(2):
                                    h = 2 * j + jj
                                    btp = psB.tile([128, 98], f32, tag="btp")
                                    nc.tensor.transpose(
                                        btp[:], mt[:, h * 128:(h + 1) * 128],
                                        eye[:98, :98])
                                    band = bpool.tile([128, 98], bf16, tag="band")
                                    if (j + jj) % 2 == 0:
                                        nc.vector.tensor_copy(band[:], btp[:])
                                    else:
                                        nc.scalar.copy(band[:], btp[:])
                                    nc.tensor.matmul(
                                        cp[jj * 64:(jj + 1) * 64, j * blk:(j + 1) * blk],
                                        halo[:, h * 64:(h + 1) * 64],
                                        band[:, :blk],
                                        start=True, stop=True,
                                        tile_position=(0, 64 * jj),
                                    )
                            convT = opool.tile([128, NCC * blk], f32r, tag="convT")
                            nc.vector.tensor_copy(convT[:], cp[:])
                            # mm2
                            op = psB.tile([128, C], f32, tag="op")
                            for j in range(NCC):
                                nc.tensor.matmul(op[:blk, :],
                                                 convT[:, j * blk:(j + 1) * blk],
                                                 w2_t[j][:],
                                                 start=(j == 0), stop=(j == NCC - 1))
                            os = opool.tile([128, C], f32, tag="os")
                            if has_b2:
                                nc.vector.tensor_add(os[:blk, :], op[:blk, :],
                                                     b2t[:blk, :])
                            else:
                                nc.scalar.copy(os[:blk, :], op[:blk, :])
                            nc.sync.dma_start(out_d[t0:t0 + blk, b, :], os[:blk, :])

    _split_sync_waits(nc, mybir)
    return nc


def kernel(x, w1, b1, wl_w, wl_b, w2, b2):
    from concourse.bass_utils import run_bass_kernel_spmd

    x = np.asarray(x, np.float32)
    w1 = np.asarray(w1, np.float32)
    b1 = np.asarray(b1, np.float32)
    wl_w = np.asarray(wl_w, np.float32)
    wl_b = np.asarray(wl_b, np.float32)
    w2 = np.asarray(w2, np.float32)
    b2 = np.asarray(b2, np.float32)

    has_b1 = bool(np.any(b1))
    has_wlb = bool(np.any(wl_b))
    has_b2 = bool(np.any(b2))

    key = (has_b1, has_wlb, has_b2)
    if key not in _cache:
        _cache[key] = _build(*key)
    nc = _cache[key]

    eye = np.eye(128, dtype=np.float32)
    wlw_pad = np.zeros((C, HKP), np.float32)
    wlw_pad[:, :HK] = wl_w

    in_maps = []
    for c in range(NCORES):
        xs = x[:, c * BPC:(c + 1) * BPC, :]  # (T, BPC, C)
        xt = np.ascontiguousarray(xs.transpose(1, 2, 0))  # (BPC, C, T)
        in_maps.append({
            "xt": xt, "w1": w1, "wl_w": wlw_pad, "w2": w2,
            "b1": b1, "wl_b": wl_b, "b2": b2,
            "eye": eye,
        })

    res = run_bass_kernel_spmd(nc, in_maps, core_ids=list(range(NCORES)))
    out = np.empty((T, B, C), np.float32)
    for c in range(NCORES):
        out[:, c * BPC:(c + 1) * BPC, :] = res.results[c]["out"]
    return out
